# revision 1
# baseline (speedup 1.0000x reference)
"""LocalCorrelation (13x13 cost volume) Trainium2 kernel.

Full inputs z_t, z_t1: [8, 256, 128, 128] f32 -> out [8, 169, 128, 128] f32.
out[b, 13*di+dj, h, w] = sum_c z_t[b,c,h,w] * pad(z_t1)[b,c,h+di,w+dj] / 16

Sharding: data-parallel over batch, 1 batch element per NeuronCore (8 cores).

Per-core algorithm (SPMD, identical program):
  - Load z_t (scaled by 1/16) and zero-padded z_t1 into SBUF as bf16,
    channel dim on partitions (2 chunks of 128).
  - For each 8x16 output-pixel block: TensorE "block gram" matmuls
    stationary = z_t block [c,128 pixels], streaming = padded z_t1
    20x28 window [c,560] -> PSUM f32 (accumulated over 2 c-chunks).
  - PSUM -> SBUF bf16, dense DMA to DRAM scratch.
  - Shear-gather DMAs (per di) read the 13x13 tap band back into
    [di*8+dh, (w, dj)] layout -- the per-pixel diagonal offset is
    absorbed by flat DRAM addressing.
  - On-chip strided copy transposes (w,dj)->(dj,w) and casts to f32.
  - Output DMA writes [tap][h][w] with 512B runs.
"""

import numpy as np

C = 256
H = W = 128
KS = 13
KK = 169
RAD = 6
HP = WP = 140  # padded spatial
SA = 8  # block rows (stripe height)
SB = 16  # block cols
NWB = W // SB  # 8 w-blocks per stripe
NST = H // SA  # 16 stripes
WINP = SA + 2 * RAD  # 20 streamed rows per window
WINQ = SB + 2 * RAD  # 28 streamed cols per window
WIN = WINP * WINQ  # 560

_cache = {}


def _build():
    import concourse.bass as bass
    import concourse.mybir as mybir
    import concourse.tile as tile
    from concourse import bacc

    f32 = mybir.dt.float32
    bf16 = mybir.dt.bfloat16

    nc = bacc.Bacc("TRN2", target_bir_lowering=False, debug=False)
    zt_d = nc.dram_tensor("z_t", [C, H, W], f32, kind="ExternalInput")
    z1_d = nc.dram_tensor("z_t1", [C, H, W], f32, kind="ExternalInput")
    out_d = nc.dram_tensor("out", [KK, H, W], f32, kind="ExternalOutput")

    with tile.TileContext(nc) as tc:
        with tc.tile_pool(name="persist", bufs=1) as pp:
            ZT = [pp.tile([128, H * W], bf16, tag=f"zt{k}", name=f"zt{k}") for k in range(2)]
            Z1P = [pp.tile([128, HP * WP], bf16, tag=f"z1p{k}", name=f"z1p{k}") for k in range(2)]

            # ---- input load: cast f32->bf16 via SWDGE DMA ----
            # ZT is stored BLOCK-MAJOR: free index = ((si*8 + wb)*8 + dh)*16 + dw
            # so each 8x16 block's 128 pixels are contiguous (matmul stationary
            # operand requires a single free dim).
            for k in range(2):
                nc.vector.memset(Z1P[k][:, :], 0.0)

            with tc.tile_pool(name="ld", bufs=2) as ldp:
                for k in range(2):
                    for s in range(4):  # 32-row slabs
                        z1u = ldp.tile([128, 32 * W], bf16, tag="z1u", name="z1u")
                        src = z1_d.ap()[k * 128:(k + 1) * 128, s * 32:(s + 1) * 32, :]
                        nc.gpsimd.dma_start(
                            z1u.rearrange("c (h w) -> c h w", h=32), src)
                        dst = Z1P[k].rearrange("c (h w) -> c h w", h=HP)[
                            :, RAD + s * 32: RAD + (s + 1) * 32, RAD: RAD + W]
                        nc.vector.tensor_copy(dst, z1u.rearrange("c (h w) -> c h w", h=32))
                for k in range(2):
                    for s in range(4):  # 32-row slabs -> 4 stripes each
                        ztu = ldp.tile([128, 32 * W], bf16, tag="ztu", name="ztu")
                        src = zt_d.ap()[k * 128:(k + 1) * 128, s * 32:(s + 1) * 32, :]
                        nc.gpsimd.dma_start(
                            ztu.rearrange("c (h w) -> c h w", h=32), src)
                        for sl in range(4):
                            si_g = s * 4 + sl
                            srcv = ztu.rearrange(
                                "c (h wb dw) -> c wb h dw", h=32, wb=NWB)[
                                :, :, sl * SA:(sl + 1) * SA, :]
                            dstv = ZT[k][:, si_g * 1024:(si_g + 1) * 1024].rearrange(
                                "c (wb dh dw) -> c wb dh dw", wb=NWB, dh=SA)
                            nc.vector.tensor_copy(dstv, srcv)
            for k in range(2):
                nc.vector.tensor_scalar_mul(ZT[k][:, :], ZT[k][:, :], 1.0 / 16.0)

            # ---- main loop ----
            with (
                tc.tile_pool(name="xbp", bufs=2) as xbp,
                tc.tile_pool(name="o2p", bufs=2) as o2p,
                tc.tile_pool(name="o3p", bufs=2) as o3p,
                tc.tile_pool(name="psp", bufs=2, space="PSUM") as psp,
                tc.tile_pool(name="scrp", bufs=2, space="DRAM") as scrp,
            ):
                for si in range(NST):
                    h0 = si * SA
                    scr = scrp.tile([NWB, 128, WIN], bf16, tag="scr", name="scr")
                    xb = xbp.tile([128, NWB * WIN], bf16, tag="xb", name="xb")
                    for wb in range(NWB):
                        w0 = wb * SB
                        ps = [psp.tile([128, 280], f32, tag=f"ps{i}", name=f"ps{i}")
                              for i in range(2)]
                        for k in range(2):
                            blk = si * NWB + wb
                            lhsT = ZT[k][:, blk * 128:(blk + 1) * 128]
                            for half in range(2):
                                rhs = Z1P[k].rearrange("c (h w) -> c h w", h=HP)[
                                    :, h0 + 10 * half: h0 + 10 * (half + 1),
                                    w0:w0 + WINQ]
                                nc.tensor.matmul(ps[half][:, :], lhsT, rhs,
                                                 start=(k == 0), stop=(k == 1))
                        for half in range(2):
                            dst = xb[:, wb * WIN + half * 280: wb * WIN + (half + 1) * 280]
                            if wb % 2 == 0:
                                nc.scalar.copy(dst, ps[half][:, :])
                            else:
                                nc.vector.tensor_copy(dst, ps[half][:, :])

                    # dense scratch write (1120B runs per (m, wb))
                    scr_w = bass.AP(scr.tensor, 0, [[WIN, 128], [128 * WIN, NWB], [1, WIN]])
                    nc.sync.dma_start(scr_w, xb.rearrange("p (wb s) -> p wb s", wb=NWB))

                    # shear-gather: per (di, wb), absorb diagonal in DRAM strides
                    # (DMA APs are limited to 3 dims)
                    o2 = o2p.tile([104, 128 * KS], bf16, tag="o2", name="o2")
                    for di in range(KS):
                        for wb in range(NWB):
                            src = bass.AP(scr.tensor, di * WINQ + wb * 128 * WIN,
                                          [[SB * WIN + WINQ, SA],
                                           [WIN + 1, SB],
                                           [1, KS]])
                            dst = o2[di * SA:(di + 1) * SA,
                                     wb * SB * KS:(wb + 1) * SB * KS].rearrange(
                                "p (dw dj) -> p dw dj", dw=SB)
                            nc.sync.dma_start(dst, src)

                    # (w, dj) -> (dj, w) transpose + cast to f32
                    o3 = o3p.tile([104, KS * W], f32, tag="o3", name="o3")
                    src_t = o2.rearrange("p (w dj) -> p dj w", dj=KS)
                    dst_t = o3.rearrange("p (dj w) -> p dj w", dj=KS)
                    if si % 2 == 0:
                        nc.vector.tensor_copy(dst_t, src_t)
                    else:
                        nc.scalar.copy(dst_t, src_t)

                    # final output write: 512B runs
                    for di in range(KS):
                        srcw = o3[di * SA:(di + 1) * SA, :].rearrange(
                            "p (dj w) -> p dj w", dj=KS)
                        dstw = bass.AP(out_d, di * KS * H * W + h0 * W,
                                       [[W, SA], [H * W, KS], [1, W]])
                        nc.sync.dma_start(dstw, srcw)

    nc.compile()
    return nc


def _get_nc():
    if "nc" not in _cache:
        _cache["nc"] = _build()
    return _cache["nc"]


def kernel(z_t: np.ndarray, z_t1: np.ndarray) -> np.ndarray:
    from concourse.bass_utils import run_bass_kernel_spmd

    nc = _get_nc()
    z_t = np.ascontiguousarray(z_t, dtype=np.float32)
    z_t1 = np.ascontiguousarray(z_t1, dtype=np.float32)
    B = z_t.shape[0]
    in_maps = [{"z_t": z_t[i], "z_t1": z_t1[i]} for i in range(B)]
    res = run_bass_kernel_spmd(nc, in_maps, core_ids=list(range(B)))
    return np.stack([res.results[i]["out"] for i in range(B)], axis=0)



# revision 2
# speedup vs baseline: 1.3337x; 1.3337x over previous
"""LocalCorrelation (13x13 cost volume) Trainium2 kernel, v2.

Full inputs z_t, z_t1: [8, 256, 128, 128] f32 -> out [8, 169, 128, 128] f32.
out[b, 13*di+dj, h, w] = sum_c z_t[b,c,h,w] * pad(z_t1)[b,c,h+di,w+dj] / 16

Sharding: data-parallel over batch, 1 batch element per NeuronCore (8 cores).

Per-core v2 pipeline (all on-chip, no DRAM scratch):
  - z_t1 zero-padded into SBUF bf16 (channel on partitions, 2 chunks).
    z_t streamed in 32-row quarter tiles (bf16, row-major).
  - Per 16-row stripe, per 8-col block: TensorE block-gram matmuls
    stationary = z_t block [c, 16x8 pixels], moving = z_t1 28x20 window.
    PSUM f32 -> SBUF bf16 (xb).
  - 16 band DMAs (SBUF->SBUF, one per dh): read each pixel's 260-element
    window band (absorbs the dh*20 shear via per-DMA offsets) and
    repartition pixels to p2 = dw*16+dh so each 16-partition group has
    constant dw.
  - One gpsimd ap_gather per stripe extracts the 169 taps per pixel
    (absorbs the per-group dw shear via constant int16 indices).
  - TensorE permutation-matmuls (stationary = gathered taps, moving =
    constant perm matrix with 1/16 scale) transpose taps onto partitions.
  - Output DMAs write [tap][h][w] with 2KB runs.
"""

import numpy as np

C = 256
H = W = 128
KS = 13
KK = 169
RAD = 6
HP = WP = 140          # padded spatial
SA = 16                # stripe rows
SB = 8                 # block cols
NST = H // SA          # 8 stripes
NWB = W // SB          # 16 w-blocks
WINP = SA + 2 * RAD    # 28 window rows
WINQ = SB + 2 * RAD    # 20 window cols
WIN = WINP * WINQ      # 560
BAND = 260             # 12*20 + 12 + 8 = per-pixel tap band (+dw slack)
NE = NWB * BAND        # 4160 gather source elems / partition
NI = NWB * KK          # 2704 gather indices
FS = NWB * WIN         # 8960 xb free size
TCA = 117              # tap chunk A (di 0..8)
TCB = KK - TCA         # 52

_cache = {}


def _consts():
    # gather indices: group g = dw; i = wb*169 + di*13 + dj (lex order)
    idx = np.zeros((128, NI // 16), np.int16)
    for g in range(8):
        for i in range(NI):
            wb, r = divmod(i, KK)
            di, dj = divmod(r, KS)
            v = wb * BAND + di * WINQ + dj + g
            idx[g * 16 + (i % 16), i // 16] = v
    # permutation matrix: p2 = dw*16+dh -> pixel n = dh*8+dw, with 1/16 scale
    import jax.numpy as jnp
    perm = np.zeros((128, 128), np.float32)
    for p2 in range(128):
        dw, dh = divmod(p2, 16)
        perm[p2, dh * 8 + dw] = 1.0 / 16.0
    perm_bf = np.asarray(jnp.asarray(perm, jnp.bfloat16))
    return idx, perm_bf


def _build():
    import concourse.bass as bass
    import concourse.mybir as mybir
    import concourse.tile as tile
    from concourse import bacc

    f32 = mybir.dt.float32
    bf16 = mybir.dt.bfloat16
    i16 = mybir.dt.int16

    nc = bacc.Bacc("TRN2", target_bir_lowering=False, debug=False)
    zt_d = nc.dram_tensor("z_t", [C, H, W], f32, kind="ExternalInput")
    z1_d = nc.dram_tensor("z_t1", [C, H, W], f32, kind="ExternalInput")
    idx_d = nc.dram_tensor("idx", [128, NI // 16], i16, kind="ExternalInput")
    perm_d = nc.dram_tensor("perm", [128, 128], f32, kind="ExternalInput")
    out_d = nc.dram_tensor("out", [KK, H, W], f32, kind="ExternalOutput")

    with tile.TileContext(nc) as tc:
        with tc.tile_pool(name="persist", bufs=1) as pp:
            Z1P = [pp.tile([128, HP * WP], bf16, tag=f"z1p{k}", name=f"z1p{k}")
                   for k in range(2)]
            idxs = pp.tile([128, NI // 16], i16, tag="idx", name="idx")
            permf = pp.tile([128, 128], f32, tag="permf", name="permf")
            perm = pp.tile([128, 128], bf16, tag="perm", name="perm")

            nc.sync.dma_start(idxs[:, :], idx_d.ap()[:, :])
            nc.sync.dma_start(permf[:, :], perm_d.ap()[:, :])
            nc.vector.tensor_copy(perm[:, :], permf[:, :])

            # z1 padded halo memsets (top/bottom rows, left/right cols)
            for k in range(2):
                zv = Z1P[k].rearrange("c (h w) -> c h w", h=HP)
                nc.vector.memset(zv[:, 0:RAD, :], 0.0)
                nc.vector.memset(zv[:, HP - RAD:HP, :], 0.0)
                nc.vector.memset(zv[:, RAD:HP - RAD, 0:RAD], 0.0)
                nc.vector.memset(zv[:, RAD:HP - RAD, WP - RAD:WP], 0.0)
            # z1 interior load with f32->bf16 cast (gpsimd swdge)
            for k in range(2):
                zv = Z1P[k].rearrange("c (h w) -> c h w", h=HP)
                for s in range(4):
                    src = z1_d.ap()[k * 128:(k + 1) * 128, s * 32:(s + 1) * 32, :]
                    dst = zv[:, RAD + s * 32: RAD + (s + 1) * 32, RAD: RAD + W]
                    nc.gpsimd.dma_start(dst, src)

            with (
                tc.tile_pool(name="ztup", bufs=2) as ztup,
                tc.tile_pool(name="ztp", bufs=2) as ztp,
                tc.tile_pool(name="xbp", bufs=2) as xbp,
                tc.tile_pool(name="o5p", bufs=1) as o5p,
                tc.tile_pool(name="o5fp", bufs=1) as o5fp,
                tc.tile_pool(name="o6fp", bufs=1) as o6fp,
                tc.tile_pool(name="o6p", bufs=1) as o6p,
                tc.tile_pool(name="obp", bufs=1) as obp,
                tc.tile_pool(name="psp", bufs=2, space="PSUM") as psp,
                tc.tile_pool(name="ptp", bufs=2, space="PSUM") as ptp,
            ):
                ztb = {}

                def load_zt_stripe(s):
                    # DMA 16-row slab (cast), then rearrange to block-major:
                    # free = wb*128 + dh*8 + dw so each block's stationary
                    # operand is one contiguous 128-elem free dim.
                    t = [ztp.tile([128, SA * W], bf16, tag=f"ztb{k}",
                                  name=f"ztb{k}_{s}") for k in range(2)]
                    for k in range(2):
                        ztu = ztup.tile([128, SA * W], bf16, tag="ztu",
                                        name=f"ztu{k}_{s}")
                        src = zt_d.ap()[k * 128:(k + 1) * 128,
                                        s * SA:(s + 1) * SA, :]
                        nc.gpsimd.dma_start(
                            ztu.rearrange("c (h w) -> c h w", h=SA), src)
                        srcv = bass.AP(ztu.tensor, 0,
                                       [[SA * W, 128], [8, NWB], [W, SA], [1, SB]])
                        dstv = bass.AP(t[k].tensor, 0,
                                       [[SA * W, 128], [128, NWB], [SB, SA], [1, SB]])
                        if k == 0:
                            nc.vector.tensor_copy(dstv, srcv)
                        else:
                            nc.scalar.copy(dstv, srcv)
                    ztb[s] = t

                load_zt_stripe(0)

                for si in range(NST):
                    if si + 1 < NST:
                        load_zt_stripe(si + 1)

                    xb = xbp.tile([128, FS], bf16, tag="xb", name="xb")
                    for wb in range(NWB):
                        ps = [psp.tile([128, 280], f32, tag=f"ps{i}",
                                       name=f"ps{i}") for i in range(2)]
                        for k in range(2):
                            lhsT = ztb[si][k][:, wb * 128:(wb + 1) * 128]
                            for half in range(2):
                                rhs = Z1P[k].rearrange(
                                    "c (h w) -> c h w", h=HP)[
                                    :, si * SA + 14 * half: si * SA + 14 * (half + 1),
                                    wb * SB: wb * SB + WINQ]
                                nc.tensor.matmul(ps[half][:, :], lhsT, rhs,
                                                 start=(k == 0), stop=(k == 1))
                        for half in range(2):
                            dst = xb[:, wb * WIN + half * 280:
                                     wb * WIN + (half + 1) * 280]
                            if wb % 2 == 0:
                                nc.scalar.copy(dst, ps[half][:, :])
                            else:
                                nc.vector.tensor_copy(dst, ps[half][:, :])

                    # band + repartition DMAs: p = dh*8+dw -> p2 = dw*16+dh
                    o5 = o5p.tile([128, NE], bf16, tag="o5", name="o5")
                    for dh in range(SA):
                        src_ap = bass.AP(xb.tensor, dh * 8 * FS + dh * WINQ,
                                         [[FS, 8], [WIN, NWB], [1, BAND]])
                        dst_ap = bass.AP(o5.tensor, dh * NE,
                                         [[16 * NE, 8], [BAND, NWB], [1, BAND]])
                        nc.sync.dma_start(dst_ap, src_ap)

                    # cast to f32 for ap_gather (d=1 requires 4-byte dtype)
                    o5f = o5fp.tile([128, NE], f32, tag="o5f", name="o5f")
                    nc.vector.tensor_copy(o5f[:, :], o5[:, :])

                    o6f = o6fp.tile([128, NI], f32, tag="o6f", name="o6f")
                    nc.gpsimd.ap_gather(o6f[:, :], o5f[:, :], idxs[:, :],
                                        channels=128, num_elems=NE, d=1,
                                        num_idxs=NI)
                    o6 = o6p.tile([128, NI], bf16, tag="o6", name="o6")
                    nc.vector.tensor_copy(o6[:, :], o6f[:, :])

                    # tap transpose: psum_t[t, dh*8+dw] via perm matmul
                    ob = [obp.tile([128, SA * W], f32, tag=f"ob{i}",
                                   name=f"ob{i}") for i in range(2)]
                    for wb in range(NWB):
                        pt = [ptp.tile([128, 128], f32, tag=f"pt{i}",
                                       name=f"pt{i}") for i in range(2)]
                        lhsA = bass.AP(o6.tensor, wb * KK,
                                       [[NI, 128], [1, TCA]])
                        lhsB = bass.AP(o6.tensor, wb * KK + TCA,
                                       [[NI, 128], [1, TCB]])
                        nc.tensor.matmul(pt[0][:TCA, :], lhsA, perm[:, :],
                                         start=True, stop=True)
                        nc.tensor.matmul(pt[1][:TCB, :], lhsB, perm[:, :],
                                         start=True, stop=True)
                        for i, tc_n in ((0, TCA), (1, TCB)):
                            src = bass.AP(pt[i].tensor, 0,
                                          [[128, tc_n], [SB, SA], [1, SB]])
                            dst = bass.AP(ob[i].tensor, wb * SB,
                                          [[SA * W, tc_n], [W, SA], [1, SB]])
                            if wb % 2 == 0:
                                nc.vector.tensor_copy(dst, src)
                            else:
                                nc.scalar.copy(dst, src)

                    # output: 8 DMAs (4 dh-quarters x 2 tap chunks), 2KB runs
                    for i, (tc_n, tbase) in enumerate(((TCA, 0), (TCB, TCA))):
                        for dq in range(4):
                            src = bass.AP(ob[i].tensor, dq * 4 * W,
                                          [[SA * W, tc_n], [1, 4 * W]])
                            dst = bass.AP(out_d,
                                          tbase * H * W + (si * SA + dq * 4) * W,
                                          [[H * W, tc_n], [1, 4 * W]])
                            nc.scalar.dma_start(dst, src)

    nc.compile()
    return nc


def _get_nc():
    if "nc" not in _cache:
        _cache["nc"] = _build()
    return _cache["nc"]


def kernel(z_t: np.ndarray, z_t1: np.ndarray) -> np.ndarray:
    from concourse.bass_utils import run_bass_kernel_spmd

    nc = _get_nc()
    z_t = np.ascontiguousarray(z_t, dtype=np.float32)
    z_t1 = np.ascontiguousarray(z_t1, dtype=np.float32)
    idx, perm_bf = _consts()
    perm_f = perm_bf.astype(np.float32)
    B = z_t.shape[0]
    in_maps = [{"z_t": z_t[i], "z_t1": z_t1[i], "idx": idx, "perm": perm_f}
               for i in range(B)]
    res = run_bass_kernel_spmd(nc, in_maps, core_ids=list(range(B)))
    return np.stack([res.results[i]["out"] for i in range(B)], axis=0)


# revision 3
# speedup vs baseline: 1.4897x; 1.1170x over previous
"""LocalCorrelation (13x13 cost volume) Trainium2 kernel, v2.

Full inputs z_t, z_t1: [8, 256, 128, 128] f32 -> out [8, 169, 128, 128] f32.
out[b, 13*di+dj, h, w] = sum_c z_t[b,c,h,w] * pad(z_t1)[b,c,h+di,w+dj] / 16

Sharding: data-parallel over batch, 1 batch element per NeuronCore (8 cores).

Per-core v2 pipeline (all on-chip, no DRAM scratch), software-pipelined
two stripes deep so TensorE streams back-to-back (p-state ramp):
  stage A (stripe si):   block-gram matmuls -> PSUM -> xb (bf16),
                         16 band DMAs (SBUF->SBUF, absorb dh*20 shear,
                         repartition p=dh*8+dw -> p2=dw*16+dh)
  stage B (stripe si-1): cast -> ap_gather (absorbs per-group dw shear)
                         -> cast -> TensorE perm-matmul tap transpose
                         -> obuf -> output DMAs (2KB runs)
"""

import numpy as np

C = 256
H = W = 128
KS = 13
KK = 169
RAD = 6
HP = WP = 140          # padded spatial
SA = 16                # stripe rows
SB = 8                 # block cols
NST = H // SA          # 8 stripes
NWB = W // SB          # 16 w-blocks
WINP = SA + 2 * RAD    # 28 window rows
WINQ = SB + 2 * RAD    # 20 window cols
WIN = WINP * WINQ      # 560
BAND = 260             # 12*20 + 12 + 8: per-pixel tap band (+dw slack)
NE = NWB * BAND        # 4160 gather source elems / partition
NI = NWB * KK          # 2704 real gather indices
NI2 = 2880             # padded to 16*180 so lhsB can be 117 wide
TCA = 117              # tap chunk A size (and padded B stationary width)
TCB = KK - TCA         # 52 real taps in chunk B

_cache = {}


def _consts():
    # gather indices: group g = dw; i = wb*169 + di*13 + dj (lex order)
    idx = np.zeros((128, NI2 // 16), np.int16)
    for g in range(8):
        for i in range(NI2):
            if i < NI:
                wb, r = divmod(i, KK)
                di, dj = divmod(r, KS)
                v = wb * BAND + di * WINQ + dj + g
            else:
                v = 0
            idx[g * 16 + (i % 16), i // 16] = v
    # permutation matrix: p2 = dw*16+dh -> pixel n = dh*8+dw, with 1/16 scale
    import jax.numpy as jnp
    perm = np.zeros((128, 128), np.float32)
    for p2 in range(128):
        dw, dh = divmod(p2, 16)
        perm[p2, dh * 8 + dw] = 1.0 / 16.0
    perm_bf = np.asarray(jnp.asarray(perm, jnp.bfloat16))
    return idx, perm_bf


def _build():
    import concourse.bass as bass
    import concourse.mybir as mybir
    import concourse.tile as tile
    from concourse import bacc

    f32 = mybir.dt.float32
    bf16 = mybir.dt.bfloat16
    i16 = mybir.dt.int16

    nc = bacc.Bacc("TRN2", target_bir_lowering=False, debug=False)
    zt_d = nc.dram_tensor("z_t", [C, H, W], f32, kind="ExternalInput")
    z1_d = nc.dram_tensor("z_t1", [C, H, W], f32, kind="ExternalInput")
    idx_d = nc.dram_tensor("idx", [128, NI2 // 16], i16, kind="ExternalInput")
    perm_d = nc.dram_tensor("perm", [128, 128], f32, kind="ExternalInput")
    out_d = nc.dram_tensor("out", [KK, H, W], f32, kind="ExternalOutput")

    with tile.TileContext(nc) as tc:
        with tc.tile_pool(name="persist", bufs=1) as pp:
            Z1P = [pp.tile([128, HP * WP], bf16, tag=f"z1p{k}", name=f"z1p{k}")
                   for k in range(2)]
            idxs = pp.tile([128, NI2 // 16], i16, tag="idx", name="idx")
            permf = pp.tile([128, 128], f32, tag="permf", name="permf")
            perm = pp.tile([128, 128], bf16, tag="perm", name="perm")

            nc.sync.dma_start(idxs[:, :], idx_d.ap()[:, :])
            nc.sync.dma_start(permf[:, :], perm_d.ap()[:, :])
            nc.vector.tensor_copy(perm[:, :], permf[:, :])

            # z1 padded halo memsets (top/bottom rows, left/right cols)
            for k in range(2):
                zv = Z1P[k].rearrange("c (h w) -> c h w", h=HP)
                nc.vector.memset(zv[:, 0:RAD, :], 0.0)
                nc.vector.memset(zv[:, HP - RAD:HP, :], 0.0)
                nc.vector.memset(zv[:, RAD:HP - RAD, 0:RAD], 0.0)
                nc.vector.memset(zv[:, RAD:HP - RAD, WP - RAD:WP], 0.0)
            # z1 interior load with f32->bf16 cast (gpsimd swdge)
            for k in range(2):
                zv = Z1P[k].rearrange("c (h w) -> c h w", h=HP)
                for s in range(4):
                    src = z1_d.ap()[k * 128:(k + 1) * 128, s * 32:(s + 1) * 32, :]
                    dst = zv[:, RAD + s * 32: RAD + (s + 1) * 32, RAD: RAD + W]
                    nc.gpsimd.dma_start(dst, src)

            with (
                tc.tile_pool(name="ztup", bufs=1) as ztup,
                tc.tile_pool(name="ztp", bufs=2) as ztp,
                tc.tile_pool(name="xbp", bufs=2) as xbp,
                tc.tile_pool(name="o5p", bufs=2) as o5p,
                tc.tile_pool(name="o5fp", bufs=1) as o5fp,
                tc.tile_pool(name="o6fp", bufs=1) as o6fp,
                tc.tile_pool(name="o6p", bufs=1) as o6p,
                tc.tile_pool(name="obp", bufs=1) as obp,
                tc.tile_pool(name="psp", bufs=3, space="PSUM") as psp,
                tc.tile_pool(name="ptp", bufs=2, space="PSUM") as ptp,
            ):
                ztb = {}
                o5s = {}

                def load_zt_stripe(s):
                    # DMA 16-row slab (cast), then rearrange to block-major:
                    # free = wb*128 + dh*8 + dw so each block's stationary
                    # operand is one contiguous 128-elem free dim.
                    t = [ztp.tile([128, SA * W], bf16, tag=f"ztb{k}",
                                  name=f"ztb{k}_{s}") for k in range(2)]
                    for k in range(2):
                        ztu = ztup.tile([128, SA * W], bf16, tag="ztu",
                                        name=f"ztu{k}_{s}")
                        src = zt_d.ap()[k * 128:(k + 1) * 128,
                                        s * SA:(s + 1) * SA, :]
                        nc.gpsimd.dma_start(
                            ztu.rearrange("c (h w) -> c h w", h=SA), src)
                        srcv = bass.AP(ztu.tensor, 0,
                                       [[SA * W, 128], [8, NWB], [W, SA], [1, SB]])
                        dstv = bass.AP(t[k].tensor, 0,
                                       [[SA * W, 128], [128, NWB], [SB, SA], [1, SB]])
                        if k == 0:
                            nc.vector.tensor_copy(dstv, srcv)
                        else:
                            nc.scalar.copy(dstv, srcv)
                    ztb[s] = t

                def stage_a(si):
                    """main matmuls + psum->xb + band DMAs for stripe si"""
                    xb = xbp.tile([128, FS], bf16, tag="xb", name="xb")
                    for wb in range(NWB):
                        ps = psp.tile([128, 1024], f32, tag="ps", name="ps")
                        for k in range(2):
                            lhsT = ztb[si][k][:, wb * 128:(wb + 1) * 128]
                            for half in range(2):
                                rhs = Z1P[k].rearrange(
                                    "c (h w) -> c h w", h=HP)[
                                    :, si * SA + 14 * half: si * SA + 14 * (half + 1),
                                    wb * SB: wb * SB + WINQ]
                                nc.tensor.matmul(
                                    ps[:, half * 512: half * 512 + 280],
                                    lhsT, rhs, start=(k == 0), stop=(k == 1))
                        src = bass.AP(ps.tensor, 0,
                                      [[1024, 128], [512, 2], [1, 280]])
                        dst = bass.AP(xb.tensor, wb * WIN,
                                      [[FS, 128], [280, 2], [1, 280]])
                        if wb % 2 == 0:
                            nc.scalar.copy(dst, src)
                        else:
                            nc.vector.tensor_copy(dst, src)

                    # band + repartition DMAs: p = dh*8+dw -> p2 = dw*16+dh
                    o5 = o5p.tile([128, NE], bf16, tag="o5", name="o5")
                    for dh in range(SA):
                        src_ap = bass.AP(xb.tensor, dh * 8 * FS + dh * WINQ,
                                         [[FS, 8], [WIN, NWB], [1, BAND]])
                        dst_ap = bass.AP(o5.tensor, dh * NE,
                                         [[16 * NE, 8], [BAND, NWB], [1, BAND]])
                        nc.sync.dma_start(dst_ap, src_ap)
                    o5s[si] = o5

                def stage_b(si):
                    """gather + tap transpose + output for stripe si"""
                    o5 = o5s.pop(si)
                    o5f = o5fp.tile([128, NE], f32, tag="o5f", name="o5f")
                    nc.vector.tensor_copy(o5f[:, :], o5[:, :])

                    o6f = o6fp.tile([128, NI2], f32, tag="o6f", name="o6f")
                    nc.gpsimd.ap_gather(o6f[:, :], o5f[:, :], idxs[:, :],
                                        channels=128, num_elems=NE, d=1,
                                        num_idxs=NI2)
                    o6 = o6p.tile([128, NI2], bf16, tag="o6", name="o6")
                    nc.vector.tensor_copy(o6[:, :], o6f[:, :])

                    # tap transpose: psum_t[t, dh*8+dw] via perm matmul
                    ob = obp.tile([128, 2 * SA * W], f32, tag="ob", name="ob")
                    for wb in range(NWB):
                        pt = ptp.tile([128, 256], f32, tag="pt", name="pt")
                        lhsA = bass.AP(o6.tensor, wb * KK,
                                       [[NI2, 128], [1, TCA]])
                        lhsB = bass.AP(o6.tensor, wb * KK + TCA,
                                       [[NI2, 128], [1, TCA]])
                        nc.tensor.matmul(pt[:TCA, 0:128], lhsA, perm[:, :],
                                         start=True, stop=True)
                        nc.tensor.matmul(pt[:TCA, 128:256], lhsB, perm[:, :],
                                         start=True, stop=True)
                        src = bass.AP(pt.tensor, 0,
                                      [[256, TCA], [128, 2], [8, SA], [1, SB]])
                        dst = bass.AP(ob.tensor, wb * SB,
                                      [[2 * SA * W, TCA], [SA * W, 2],
                                       [W, SA], [1, SB]])
                        if wb % 2 == 0:
                            nc.vector.tensor_copy(dst, src)
                        else:
                            nc.scalar.copy(dst, src)

                    # output: 8 DMAs (4 dh-quarters x 2 tap chunks), 2KB runs
                    for i, (tc_n, tbase, obase) in enumerate(
                            ((TCA, 0, 0), (TCB, TCA, SA * W))):
                        for dq in range(4):
                            src = bass.AP(ob.tensor, obase + dq * 4 * W,
                                          [[2 * SA * W, tc_n], [1, 4 * W]])
                            dst = bass.AP(out_d,
                                          tbase * H * W + (si * SA + dq * 4) * W,
                                          [[H * W, tc_n], [1, 4 * W]])
                            nc.scalar.dma_start(dst, src)

                load_zt_stripe(0)
                for si in range(NST):
                    if si + 1 < NST:
                        load_zt_stripe(si + 1)
                    stage_a(si)
                    if si > 0:
                        stage_b(si - 1)
                stage_b(NST - 1)

    nc.compile()
    return nc


FS = NWB * WIN  # 8960 xb free size


def _get_nc():
    if "nc" not in _cache:
        _cache["nc"] = _build()
    return _cache["nc"]


def kernel(z_t: np.ndarray, z_t1: np.ndarray) -> np.ndarray:
    from concourse.bass_utils import run_bass_kernel_spmd

    nc = _get_nc()
    z_t = np.ascontiguousarray(z_t, dtype=np.float32)
    z_t1 = np.ascontiguousarray(z_t1, dtype=np.float32)
    idx, perm_bf = _consts()
    perm_f = perm_bf.astype(np.float32)
    B = z_t.shape[0]
    in_maps = [{"z_t": z_t[i], "z_t1": z_t1[i], "idx": idx, "perm": perm_f}
               for i in range(B)]
    res = run_bass_kernel_spmd(nc, in_maps, core_ids=list(range(B)))
    return np.stack([res.results[i]["out"] for i in range(B)], axis=0)


# revision 4
# speedup vs baseline: 2.7727x; 1.8612x over previous
"""LocalCorrelation (13x13 cost volume) Trainium2 kernel, v2.

Full inputs z_t, z_t1: [8, 256, 128, 128] f32 -> out [8, 169, 128, 128] f32.
out[b, 13*di+dj, h, w] = sum_c z_t[b,c,h,w] * pad(z_t1)[b,c,h+di,w+dj] / 16

Sharding: data-parallel over batch, 1 batch element per NeuronCore (8 cores).

Per-core v2 pipeline (all on-chip, no DRAM scratch), software-pipelined
two stripes deep so TensorE streams back-to-back (p-state ramp):
  stage A (stripe si):   block-gram matmuls -> PSUM -> xb (bf16);
                         hop1: 16 band DMAs (SBUF->SBUF, absorb the
                         per-row dh*20 window shear, repartition
                         p=dh*8+dw -> p2=dw*16+dh);
                         hop2: 8 DMAs (absorb the per-col +dw shear,
                         constant offset per dw group).
  stage B (stripe si-1): one strided copy extracts the 169 taps/pixel,
                         TensorE perm-matmul transposes taps onto
                         partitions (with 1/16 scale), obuf assembly,
                         output DMAs with 2KB runs.
"""

import numpy as np

C = 256
H = W = 128
KS = 13
KK = 169
RAD = 6
HP = WP = 140          # padded spatial
SA = 16                # stripe rows
SB = 8                 # block cols
NST = H // SA          # 8 stripes
NWB = W // SB          # 16 w-blocks
WINP = SA + 2 * RAD    # 28 window rows
WINQ = SB + 2 * RAD    # 20 window cols
WIN = WINP * WINQ      # 560
BAND = 260             # 12*20 + 12 + 8: per-pixel tap band (+dw slack)
EB = 253               # dw-aligned band (12*20 + 12 + 1)
NE = NWB * BAND        # 4160 o5 elems / partition
NEB = NWB * EB         # 4048 o5b elems / partition
NI2 = 2880             # o6 free, padded past 2704 for lhsB reads
TCA = 117              # tap chunk A size (and padded B stationary width)
TCB = KK - TCA         # 52 real taps in chunk B
FS = NWB * WIN         # 8960 xb free size

_cache = {}


def _consts():
    # permutation matrix: p2 = dw*16+dh -> pixel n = dh*8+dw, with 1/16 scale
    perm = np.zeros((128, 128), np.float32)
    for p2 in range(128):
        dw, dh = divmod(p2, 16)
        perm[p2, dh * 8 + dw] = 1.0 / 16.0
    return perm


def _build():
    import concourse.bass as bass
    import concourse.mybir as mybir
    import concourse.tile as tile
    from concourse import bacc

    f32 = mybir.dt.float32
    bf16 = mybir.dt.bfloat16

    nc = bacc.Bacc("TRN2", target_bir_lowering=False, debug=False)
    zt_d = nc.dram_tensor("z_t", [C, H, W], f32, kind="ExternalInput")
    z1_d = nc.dram_tensor("z_t1", [C, H, W], f32, kind="ExternalInput")
    perm_d = nc.dram_tensor("perm", [128, 128], f32, kind="ExternalInput")
    out_d = nc.dram_tensor("out", [KK, H, W], f32, kind="ExternalOutput")

    with tile.TileContext(nc) as tc:
        with tc.tile_pool(name="persist", bufs=1) as pp:
            Z1P = [pp.tile([128, HP * WP], bf16, tag=f"z1p{k}", name=f"z1p{k}")
                   for k in range(2)]
            permf = pp.tile([128, 128], f32, tag="permf", name="permf")
            perm = pp.tile([128, 128], bf16, tag="perm", name="perm")

            nc.sync.dma_start(permf[:, :], perm_d.ap()[:, :])
            nc.vector.tensor_copy(perm[:, :], permf[:, :])

            # z1 padded halo memsets (top/bottom rows, left/right cols)
            for k in range(2):
                zv = Z1P[k].rearrange("c (h w) -> c h w", h=HP)
                nc.vector.memset(zv[:, 0:RAD, :], 0.0)
                nc.vector.memset(zv[:, HP - RAD:HP, :], 0.0)
                nc.vector.memset(zv[:, RAD:HP - RAD, 0:RAD], 0.0)
                nc.vector.memset(zv[:, RAD:HP - RAD, WP - RAD:WP], 0.0)
            # z1 interior load with f32->bf16 cast (gpsimd swdge)
            for k in range(2):
                zv = Z1P[k].rearrange("c (h w) -> c h w", h=HP)
                for s in range(4):
                    src = z1_d.ap()[k * 128:(k + 1) * 128, s * 32:(s + 1) * 32, :]
                    dst = zv[:, RAD + s * 32: RAD + (s + 1) * 32, RAD: RAD + W]
                    nc.gpsimd.dma_start(dst, src)

            with (
                tc.tile_pool(name="ztup", bufs=2) as ztup,
                tc.tile_pool(name="ztp", bufs=2) as ztp,
                tc.tile_pool(name="xbp", bufs=2) as xbp,
                tc.tile_pool(name="o5p", bufs=2) as o5p,
                tc.tile_pool(name="o5bp", bufs=2) as o5bp,
                tc.tile_pool(name="o6p", bufs=2) as o6p,
                tc.tile_pool(name="obp", bufs=1) as obp,
                tc.tile_pool(name="psp", bufs=3, space="PSUM") as psp,
                tc.tile_pool(name="ptp", bufs=2, space="PSUM") as ptp,
            ):
                ztb = {}
                o5bs = {}

                def load_zt_stripe(s):
                    # DMA 16-row slab (cast), then rearrange to block-major:
                    # free = wb*128 + dh*8 + dw so each block's stationary
                    # operand is one contiguous 128-elem free dim.
                    t = [ztp.tile([128, SA * W], bf16, tag=f"ztb{k}",
                                  name=f"ztb{k}_{s}") for k in range(2)]
                    for k in range(2):
                        ztu = ztup.tile([128, SA * W], bf16, tag="ztu",
                                        name=f"ztu{k}_{s}")
                        src = zt_d.ap()[k * 128:(k + 1) * 128,
                                        s * SA:(s + 1) * SA, :]
                        nc.gpsimd.dma_start(
                            ztu.rearrange("c (h w) -> c h w", h=SA), src)
                        srcv = bass.AP(ztu.tensor, 0,
                                       [[SA * W, 128], [8, NWB], [W, SA], [1, SB]])
                        dstv = bass.AP(t[k].tensor, 0,
                                       [[SA * W, 128], [128, NWB], [SB, SA], [1, SB]])
                        if k == 0:
                            nc.vector.tensor_copy(dstv, srcv)
                        else:
                            nc.gpsimd.tensor_copy(dstv, srcv)
                    ztb[s] = t

                def stage_a(si):
                    """main matmuls + psum->xb + band hop1/hop2 DMAs"""
                    xb = xbp.tile([128, FS], bf16, tag="xb", name="xb")
                    for wb in range(NWB):
                        ps = psp.tile([128, 1024], f32, tag="ps", name="ps")
                        for k in range(2):
                            lhsT = ztb[si][k][:, wb * 128:(wb + 1) * 128]
                            for half in range(2):
                                rhs = Z1P[k].rearrange(
                                    "c (h w) -> c h w", h=HP)[
                                    :, si * SA + 14 * half: si * SA + 14 * (half + 1),
                                    wb * SB: wb * SB + WINQ]
                                nc.tensor.matmul(
                                    ps[:, half * 512: half * 512 + 280],
                                    lhsT, rhs, start=(k == 0), stop=(k == 1))
                        src = bass.AP(ps.tensor, 0,
                                      [[1024, 128], [512, 2], [1, 280]])
                        dst = bass.AP(xb.tensor, wb * WIN,
                                      [[FS, 128], [280, 2], [1, 280]])
                        if wb % 2 == 0:
                            nc.scalar.copy(dst, src)
                        else:
                            nc.vector.tensor_copy(dst, src)

                    # hop1: band + repartition: p = dh*8+dw -> p2 = dw*16+dh
                    o5 = o5p.tile([128, NE], bf16, tag="o5", name="o5")
                    for dh in range(SA):
                        src_ap = bass.AP(xb.tensor, dh * 8 * FS + dh * WINQ,
                                         [[FS, 8], [WIN, NWB], [1, BAND]])
                        dst_ap = bass.AP(o5.tensor, dh * NE,
                                         [[16 * NE, 8], [BAND, NWB], [1, BAND]])
                        nc.sync.dma_start(dst_ap, src_ap)
                    # hop2: per dw group, shift band start by dw
                    o5b = o5bp.tile([128, NEB], bf16, tag="o5b", name="o5b")
                    for dw in range(8):
                        src_ap = bass.AP(o5.tensor, dw * 16 * NE + dw,
                                         [[NE, 16], [BAND, NWB], [1, EB]])
                        dst_ap = bass.AP(o5b.tensor, dw * 16 * NEB,
                                         [[NEB, 16], [EB, NWB], [1, EB]])
                        if dw % 2 == 0:
                            nc.scalar.dma_start(dst_ap, src_ap)
                        else:
                            nc.sync.dma_start(dst_ap, src_ap)
                    o5bs[si] = o5b

                def stage_b(si):
                    """tap extraction + transpose + output for stripe si"""
                    o5b = o5bs.pop(si)
                    o6 = o6p.tile([128, NI2], bf16, tag="o6", name="o6")
                    # tail [2704:2880) stays zero for padded lhsB reads
                    nc.vector.memset(o6[:, NWB * KK:NI2], 0.0)
                    src = bass.AP(o5b.tensor, 0,
                                  [[NEB, 128], [EB, NWB], [WINQ, KS], [1, KS]])
                    dst = bass.AP(o6.tensor, 0,
                                  [[NI2, 128], [KK, NWB], [KS, KS], [1, KS]])
                    nc.gpsimd.tensor_copy(dst, src)

                    # tap transpose: psum_t[t, dh*8+dw] via perm matmul
                    ob = obp.tile([128, 2 * SA * W], f32, tag="ob", name="ob")
                    for wb in range(NWB):
                        pt = ptp.tile([128, 256], f32, tag="pt", name="pt")
                        lhsA = bass.AP(o6.tensor, wb * KK,
                                       [[NI2, 128], [1, TCA]])
                        lhsB = bass.AP(o6.tensor, wb * KK + TCA,
                                       [[NI2, 128], [1, TCA]])
                        nc.tensor.matmul(pt[:TCA, 0:128], lhsA, perm[:, :],
                                         start=True, stop=True)
                        nc.tensor.matmul(pt[:TCA, 128:256], lhsB, perm[:, :],
                                         start=True, stop=True)
                        src = bass.AP(pt.tensor, 0,
                                      [[256, TCA], [128, 2], [8, SA], [1, SB]])
                        dst = bass.AP(ob.tensor, wb * SB,
                                      [[2 * SA * W, TCA], [SA * W, 2],
                                       [W, SA], [1, SB]])
                        nc.vector.tensor_copy(dst, src)

                    # output: 8 DMAs (4 dh-quarters x 2 tap chunks), 2KB runs
                    for i, (tc_n, tbase, obase) in enumerate(
                            ((TCA, 0, 0), (TCB, TCA, SA * W))):
                        for dq in range(4):
                            src = bass.AP(ob.tensor, obase + dq * 4 * W,
                                          [[2 * SA * W, tc_n], [1, 4 * W]])
                            dst = bass.AP(out_d,
                                          tbase * H * W + (si * SA + dq * 4) * W,
                                          [[H * W, tc_n], [1, 4 * W]])
                            nc.scalar.dma_start(dst, src)

                load_zt_stripe(0)
                for si in range(NST):
                    if si + 1 < NST:
                        load_zt_stripe(si + 1)
                    stage_a(si)
                    if si > 0:
                        stage_b(si - 1)
                stage_b(NST - 1)

    nc.compile()
    return nc


def _get_nc():
    if "nc" not in _cache:
        _cache["nc"] = _build()
    return _cache["nc"]


def kernel(z_t: np.ndarray, z_t1: np.ndarray) -> np.ndarray:
    from concourse.bass_utils import run_bass_kernel_spmd

    nc = _get_nc()
    z_t = np.ascontiguousarray(z_t, dtype=np.float32)
    z_t1 = np.ascontiguousarray(z_t1, dtype=np.float32)
    perm_f = _consts()
    B = z_t.shape[0]
    in_maps = [{"z_t": z_t[i], "z_t1": z_t1[i], "perm": perm_f}
               for i in range(B)]
    res = run_bass_kernel_spmd(nc, in_maps, core_ids=list(range(B)))
    return np.stack([res.results[i]["out"] for i in range(B)], axis=0)


# revision 5
# speedup vs baseline: 3.4470x; 1.2432x over previous
"""LocalCorrelation (13x13 cost volume) Trainium2 kernel, v2.

Full inputs z_t, z_t1: [8, 256, 128, 128] f32 -> out [8, 169, 128, 128] f32.
out[b, 13*di+dj, h, w] = sum_c z_t[b,c,h,w] * pad(z_t1)[b,c,h+di,w+dj] / 16

Sharding: data-parallel over batch, 1 batch element per NeuronCore (8 cores).

Per-core v2 pipeline (all on-chip, no DRAM scratch), software-pipelined
two stripes deep so TensorE streams back-to-back (p-state ramp):
  stage A (stripe si):   block-gram matmuls -> PSUM -> xb (bf16);
                         hop1: 16 band DMAs (SBUF->SBUF, absorb the
                         per-row dh*20 window shear, repartition
                         p=dh*8+dw -> p2=dw*16+dh);
                         hop2: 8 DMAs (absorb the per-col +dw shear,
                         constant offset per dw group).
  stage B (stripe si-1): one strided copy extracts the 169 taps/pixel,
                         TensorE perm-matmul transposes taps onto
                         partitions (with 1/16 scale), obuf assembly,
                         output DMAs with 2KB runs.
"""

import numpy as np

C = 256
H = W = 128
KS = 13
KK = 169
RAD = 6
HP = WP = 140          # padded spatial
SA = 16                # stripe rows
SB = 8                 # block cols
NST = H // SA          # 8 stripes
NWB = W // SB          # 16 w-blocks
WINP = SA + 2 * RAD    # 28 window rows
WINQ = SB + 2 * RAD    # 20 window cols
WIN = WINP * WINQ      # 560
BAND = 260             # 12*20 + 12 + 8: per-pixel tap band (+dw slack)
EB = 253               # dw-aligned band (12*20 + 12 + 1)
NE = NWB * BAND        # 4160 o5 elems / partition
NEB = NWB * EB         # 4048 o5b elems / partition
NI2 = 2880             # o6 free, padded past 2704 for lhsB reads
TCA = 117              # tap chunk A size (and padded B stationary width)
TCB = KK - TCA         # 52 real taps in chunk B
FS = NWB * WIN         # 8960 xb free size

_cache = {}


def _consts():
    # permutation matrix: p2 = dw*16+dh -> pixel n = dh*8+dw, with 1/16 scale
    perm = np.zeros((128, 128), np.float32)
    for p2 in range(128):
        dw, dh = divmod(p2, 16)
        perm[p2, dh * 8 + dw] = 1.0 / 16.0
    return perm


def _build():
    import concourse.bass as bass
    import concourse.mybir as mybir
    import concourse.tile as tile
    from concourse import bacc

    f32 = mybir.dt.float32
    bf16 = mybir.dt.bfloat16

    nc = bacc.Bacc("TRN2", target_bir_lowering=False, debug=False)
    zt_d = nc.dram_tensor("z_t", [C, H, W], f32, kind="ExternalInput")
    z1_d = nc.dram_tensor("z_t1", [C, H, W], f32, kind="ExternalInput")
    perm_d = nc.dram_tensor("perm", [128, 128], f32, kind="ExternalInput")
    out_d = nc.dram_tensor("out", [KK, H, W], f32, kind="ExternalOutput")

    with tile.TileContext(nc) as tc:
        with tc.tile_pool(name="persist", bufs=1) as pp:
            Z1P = [pp.tile([128, HP * WP], bf16, tag=f"z1p{k}", name=f"z1p{k}")
                   for k in range(2)]
            permf = pp.tile([128, 128], f32, tag="permf", name="permf")
            perm = pp.tile([128, 128], bf16, tag="perm", name="perm")

            nc.sync.dma_start(permf[:, :], perm_d.ap()[:, :])
            nc.vector.tensor_copy(perm[:, :], permf[:, :])

            # z1 padded halo memsets (top/bottom rows, left/right cols)
            for k in range(2):
                zv = Z1P[k].rearrange("c (h w) -> c h w", h=HP)
                nc.vector.memset(zv[:, 0:RAD, :], 0.0)
                nc.vector.memset(zv[:, HP - RAD:HP, :], 0.0)
                nc.vector.memset(zv[:, RAD:HP - RAD, 0:RAD], 0.0)
                nc.vector.memset(zv[:, RAD:HP - RAD, WP - RAD:WP], 0.0)
            # z1 interior load with f32->bf16 cast (gpsimd swdge)
            for k in range(2):
                zv = Z1P[k].rearrange("c (h w) -> c h w", h=HP)
                for s in range(4):
                    src = z1_d.ap()[k * 128:(k + 1) * 128, s * 32:(s + 1) * 32, :]
                    dst = zv[:, RAD + s * 32: RAD + (s + 1) * 32, RAD: RAD + W]
                    nc.gpsimd.dma_start(dst, src)

            with (
                tc.tile_pool(name="ztup", bufs=2) as ztup,
                tc.tile_pool(name="ztp", bufs=2) as ztp,
                tc.tile_pool(name="xbp", bufs=2) as xbp,
                tc.tile_pool(name="o5p", bufs=2) as o5p,
                tc.tile_pool(name="o5bp", bufs=3) as o5bp,
                tc.tile_pool(name="o6p", bufs=2) as o6p,
                tc.tile_pool(name="obp", bufs=1) as obp,
                tc.tile_pool(name="psp", bufs=3, space="PSUM") as psp,
                tc.tile_pool(name="ptp", bufs=2, space="PSUM") as ptp,
            ):
                ztb = {}
                o5bs = {}

                def load_zt_stripe(s):
                    # DMA 16-row slab (cast), then rearrange to block-major:
                    # free = wb*128 + dh*8 + dw so each block's stationary
                    # operand is one contiguous 128-elem free dim.
                    t = [ztp.tile([128, SA * W], bf16, tag=f"ztb{k}",
                                  name=f"ztb{k}_{s}") for k in range(2)]
                    for k in range(2):
                        ztu = ztup.tile([128, SA * W], bf16, tag="ztu",
                                        name=f"ztu{k}_{s}")
                        src = zt_d.ap()[k * 128:(k + 1) * 128,
                                        s * SA:(s + 1) * SA, :]
                        nc.gpsimd.dma_start(
                            ztu.rearrange("c (h w) -> c h w", h=SA), src)
                        srcv = bass.AP(ztu.tensor, 0,
                                       [[SA * W, 128], [8, NWB], [W, SA], [1, SB]])
                        dstv = bass.AP(t[k].tensor, 0,
                                       [[SA * W, 128], [128, NWB], [SB, SA], [1, SB]])
                        if k == 0:
                            nc.vector.tensor_copy(dstv, srcv)
                        else:
                            nc.scalar.copy(dstv, srcv)
                    ztb[s] = t

                def stage_a(si):
                    """main matmuls + psum->xb + band hop1/hop2 DMAs"""
                    xb = xbp.tile([128, FS], bf16, tag="xb", name="xb")
                    for wb in range(NWB):
                        ps = psp.tile([128, 1024], f32, tag="ps", name="ps")
                        for k in range(2):
                            lhsT = ztb[si][k][:, wb * 128:(wb + 1) * 128]
                            for half in range(2):
                                rhs = Z1P[k].rearrange(
                                    "c (h w) -> c h w", h=HP)[
                                    :, si * SA + 14 * half: si * SA + 14 * (half + 1),
                                    wb * SB: wb * SB + WINQ]
                                nc.tensor.matmul(
                                    ps[:, half * 512: half * 512 + 280],
                                    lhsT, rhs, start=(k == 0), stop=(k == 1))
                        src = bass.AP(ps.tensor, 0,
                                      [[1024, 128], [512, 2], [1, 280]])
                        dst = bass.AP(xb.tensor, wb * WIN,
                                      [[FS, 128], [280, 2], [1, 280]])
                        if wb % 2 == 0:
                            nc.scalar.copy(dst, src)
                        else:
                            nc.vector.tensor_copy(dst, src)

                    # hop1: band + repartition: p = dh*8+dw -> p2 = dw*16+dh
                    o5 = o5p.tile([128, NE], bf16, tag="o5", name="o5")
                    for dh in range(SA):
                        src_ap = bass.AP(xb.tensor, dh * 8 * FS + dh * WINQ,
                                         [[FS, 8], [WIN, NWB], [1, BAND]])
                        dst_ap = bass.AP(o5.tensor, dh * NE,
                                         [[16 * NE, 8], [BAND, NWB], [1, BAND]])
                        nc.sync.dma_start(dst_ap, src_ap)
                    # hop2: per dw group, shift band start by dw
                    o5b = o5bp.tile([128, NEB], bf16, tag="o5b", name="o5b")
                    for dw in range(8):
                        src_ap = bass.AP(o5.tensor, dw * 16 * NE + dw,
                                         [[NE, 16], [BAND, NWB], [1, EB]])
                        dst_ap = bass.AP(o5b.tensor, dw * 16 * NEB,
                                         [[NEB, 16], [EB, NWB], [1, EB]])
                        if dw % 2 == 0:
                            nc.scalar.dma_start(dst_ap, src_ap)
                        else:
                            nc.sync.dma_start(dst_ap, src_ap)
                    o5bs[si] = o5b

                def stage_b(si):
                    """tap extraction + transpose + output for stripe si"""
                    o5b = o5bs.pop(si)
                    o6 = o6p.tile([128, NI2], bf16, tag="o6", name="o6")
                    # tail [2704:2880) stays zero for padded lhsB reads
                    nc.vector.memset(o6[:, NWB * KK:NI2], 0.0)
                    src = bass.AP(o5b.tensor, 0,
                                  [[NEB, 128], [EB, NWB], [WINQ, KS], [1, KS]])
                    dst = bass.AP(o6.tensor, 0,
                                  [[NI2, 128], [KK, NWB], [KS, KS], [1, KS]])
                    nc.vector.tensor_copy(dst, src)

                    # tap transpose: psum_t[t, dh*8+dw] via perm matmul
                    ob = obp.tile([128, 2 * SA * W], f32, tag="ob", name="ob")
                    for wb in range(NWB):
                        pt = ptp.tile([128, 256], f32, tag="pt", name="pt")
                        lhsA = bass.AP(o6.tensor, wb * KK,
                                       [[NI2, 128], [1, TCA]])
                        lhsB = bass.AP(o6.tensor, wb * KK + TCA,
                                       [[NI2, 128], [1, TCA]])
                        nc.tensor.matmul(pt[:TCA, 0:128], lhsA, perm[:, :],
                                         start=True, stop=True)
                        nc.tensor.matmul(pt[:TCA, 128:256], lhsB, perm[:, :],
                                         start=True, stop=True)
                        src = bass.AP(pt.tensor, 0,
                                      [[256, TCA], [128, 2], [8, SA], [1, SB]])
                        dst = bass.AP(ob.tensor, wb * SB,
                                      [[2 * SA * W, TCA], [SA * W, 2],
                                       [W, SA], [1, SB]])
                        nc.vector.tensor_copy(dst, src)

                    # output: 8 DMAs (4 dh-quarters x 2 tap chunks), 2KB runs
                    for i, (tc_n, tbase, obase) in enumerate(
                            ((TCA, 0, 0), (TCB, TCA, SA * W))):
                        for dq in range(4):
                            src = bass.AP(ob.tensor, obase + dq * 4 * W,
                                          [[2 * SA * W, tc_n], [1, 4 * W]])
                            dst = bass.AP(out_d,
                                          tbase * H * W + (si * SA + dq * 4) * W,
                                          [[H * W, tc_n], [1, 4 * W]])
                            nc.scalar.dma_start(dst, src)

                load_zt_stripe(0)
                for si in range(NST):
                    if si + 1 < NST:
                        load_zt_stripe(si + 1)
                    stage_a(si)
                    if si > 1:
                        stage_b(si - 2)
                stage_b(NST - 2)
                stage_b(NST - 1)

    nc.compile()
    return nc


def _get_nc():
    if "nc" not in _cache:
        _cache["nc"] = _build()
    return _cache["nc"]


def kernel(z_t: np.ndarray, z_t1: np.ndarray) -> np.ndarray:
    from concourse.bass_utils import run_bass_kernel_spmd

    nc = _get_nc()
    z_t = np.ascontiguousarray(z_t, dtype=np.float32)
    z_t1 = np.ascontiguousarray(z_t1, dtype=np.float32)
    perm_f = _consts()
    B = z_t.shape[0]
    in_maps = [{"z_t": z_t[i], "z_t1": z_t1[i], "perm": perm_f}
               for i in range(B)]
    res = run_bass_kernel_spmd(nc, in_maps, core_ids=list(range(B)))
    return np.stack([res.results[i]["out"] for i in range(B)], axis=0)


# revision 6
# speedup vs baseline: 4.2472x; 1.2321x over previous
"""LocalCorrelation (13x13 cost volume) Trainium2 kernel, v2.

Full inputs z_t, z_t1: [8, 256, 128, 128] f32 -> out [8, 169, 128, 128] f32.
out[b, 13*di+dj, h, w] = sum_c z_t[b,c,h,w] * pad(z_t1)[b,c,h+di,w+dj] / 16

Sharding: data-parallel over batch, 1 batch element per NeuronCore (8 cores).

Per-core v2 pipeline (all on-chip, no DRAM scratch), software-pipelined
two stripes deep so TensorE streams back-to-back (p-state ramp):
  stage A (stripe si):   block-gram matmuls -> PSUM -> xb (bf16);
                         hop1: 16 band DMAs (SBUF->SBUF, absorb the
                         per-row dh*20 window shear, repartition
                         p=dh*8+dw -> p2=dw*16+dh);
                         hop2: 8 DMAs (absorb the per-col +dw shear,
                         constant offset per dw group).
  stage B (stripe si-1): one strided copy extracts the 169 taps/pixel,
                         TensorE perm-matmul transposes taps onto
                         partitions (with 1/16 scale), obuf assembly,
                         output DMAs with 2KB runs.
"""

import numpy as np

C = 256
H = W = 128
KS = 13
KK = 169
RAD = 6
HP = WP = 140          # padded spatial
SA = 16                # stripe rows
SB = 8                 # block cols
NST = H // SA          # 8 stripes
NWB = W // SB          # 16 w-blocks
WINP = SA + 2 * RAD    # 28 window rows
WINQ = SB + 2 * RAD    # 20 window cols
WIN = WINP * WINQ      # 560
BAND = 260             # 12*20 + 12 + 8: per-pixel tap band (+dw slack)
EB = 253               # dw-aligned band (12*20 + 12 + 1)
NE = NWB * BAND        # 4160 o5 elems / partition
NEB = NWB * EB         # 4048 o5b elems / partition
NI2 = 2880             # o6 free, padded past 2704 for lhsB reads
TCA = 117              # tap chunk A size (and padded B stationary width)
TCB = KK - TCA         # 52 real taps in chunk B
FS = NWB * WIN         # 8960 xb free size

_cache = {}


def _consts():
    # permutation matrix: p2 = dw*16+dh -> pixel n = dh*8+dw, with 1/16 scale
    perm = np.zeros((128, 128), np.float32)
    for p2 in range(128):
        dw, dh = divmod(p2, 16)
        perm[p2, dh * 8 + dw] = 1.0 / 16.0
    return perm


def _build():
    import concourse.bass as bass
    import concourse.mybir as mybir
    import concourse.tile as tile
    from concourse import bacc

    f32 = mybir.dt.float32
    bf16 = mybir.dt.bfloat16

    nc = bacc.Bacc("TRN2", target_bir_lowering=False, debug=False)
    zt_d = nc.dram_tensor("z_t", [C, H, W], f32, kind="ExternalInput")
    z1_d = nc.dram_tensor("z_t1", [C, H, W], f32, kind="ExternalInput")
    perm_d = nc.dram_tensor("perm", [128, 128], f32, kind="ExternalInput")
    out_d = nc.dram_tensor("out", [KK, H, W], f32, kind="ExternalOutput")

    with tile.TileContext(nc) as tc:
        with tc.tile_pool(name="persist", bufs=1) as pp:
            Z1P = [pp.tile([128, HP * WP], bf16, tag=f"z1p{k}", name=f"z1p{k}")
                   for k in range(2)]
            permf = pp.tile([128, 128], f32, tag="permf", name="permf")
            perm = pp.tile([128, 128], bf16, tag="perm", name="perm")

            nc.sync.dma_start(permf[:, :], perm_d.ap()[:, :])
            nc.vector.tensor_copy(perm[:, :], permf[:, :])

            # z1 padded halo memsets (top/bottom rows, left/right cols)
            for k in range(2):
                zv = Z1P[k].rearrange("c (h w) -> c h w", h=HP)
                nc.vector.memset(zv[:, 0:RAD, :], 0.0)
                nc.vector.memset(zv[:, HP - RAD:HP, :], 0.0)
                nc.vector.memset(zv[:, RAD:HP - RAD, 0:RAD], 0.0)
                nc.vector.memset(zv[:, RAD:HP - RAD, WP - RAD:WP], 0.0)
            # z1 interior load with f32->bf16 cast (gpsimd swdge)
            for k in range(2):
                zv = Z1P[k].rearrange("c (h w) -> c h w", h=HP)
                for s in range(4):
                    src = z1_d.ap()[k * 128:(k + 1) * 128, s * 32:(s + 1) * 32, :]
                    dst = zv[:, RAD + s * 32: RAD + (s + 1) * 32, RAD: RAD + W]
                    nc.gpsimd.dma_start(dst, src)

            with (
                tc.tile_pool(name="ztup", bufs=2) as ztup,
                tc.tile_pool(name="ztp", bufs=2) as ztp,
                tc.tile_pool(name="xbp", bufs=2) as xbp,
                tc.tile_pool(name="o5p", bufs=2) as o5p,
                tc.tile_pool(name="o5bp", bufs=3) as o5bp,
                tc.tile_pool(name="o6p", bufs=2) as o6p,
                tc.tile_pool(name="obp", bufs=1) as obp,
                tc.tile_pool(name="psp", bufs=3, space="PSUM") as psp,
                tc.tile_pool(name="ptp", bufs=2, space="PSUM") as ptp,
            ):
                ztb = {}
                o5bs = {}

                def load_zt_stripe(s):
                    # DMA 16-row slab (cast), then rearrange to block-major:
                    # free = wb*128 + dh*8 + dw so each block's stationary
                    # operand is one contiguous 128-elem free dim.
                    t = [ztp.tile([128, SA * W], bf16, tag=f"ztb{k}",
                                  name=f"ztb{k}_{s}") for k in range(2)]
                    for k in range(2):
                        ztu = ztup.tile([128, SA * W], bf16, tag="ztu",
                                        name=f"ztu{k}_{s}")
                        src = zt_d.ap()[k * 128:(k + 1) * 128,
                                        s * SA:(s + 1) * SA, :]
                        nc.gpsimd.dma_start(
                            ztu.rearrange("c (h w) -> c h w", h=SA), src)
                        srcv = bass.AP(ztu.tensor, 0,
                                       [[SA * W, 128], [8, NWB], [W, SA], [1, SB]])
                        dstv = bass.AP(t[k].tensor, 0,
                                       [[SA * W, 128], [128, NWB], [SB, SA], [1, SB]])
                        if k == 0:
                            nc.vector.tensor_copy(dstv, srcv)
                        else:
                            nc.scalar.copy(dstv, srcv)
                    ztb[s] = t

                def stage_a(si):
                    """main matmuls + psum->xb + band hop1/hop2 DMAs"""
                    xb = xbp.tile([128, FS], bf16, tag="xb", name="xb")
                    for wb in range(NWB):
                        ps = psp.tile([128, 1024], f32, tag="ps", name="ps")
                        for k in range(2):
                            lhsT = ztb[si][k][:, wb * 128:(wb + 1) * 128]
                            for half in range(2):
                                rhs = Z1P[k].rearrange(
                                    "c (h w) -> c h w", h=HP)[
                                    :, si * SA + 14 * half: si * SA + 14 * (half + 1),
                                    wb * SB: wb * SB + WINQ]
                                nc.tensor.matmul(
                                    ps[:, half * 512: half * 512 + 280],
                                    lhsT, rhs, start=(k == 0), stop=(k == 1))
                        src = bass.AP(ps.tensor, 0,
                                      [[1024, 128], [512, 2], [1, 280]])
                        dst = bass.AP(xb.tensor, wb * WIN,
                                      [[FS, 128], [280, 2], [1, 280]])
                        if wb % 2 == 0:
                            nc.scalar.copy(dst, src)
                        else:
                            nc.vector.tensor_copy(dst, src)

                    # hop1: band + repartition: p = dh*8+dw -> p2 = dw*16+dh
                    o5 = o5p.tile([128, NE], bf16, tag="o5", name="o5")
                    for dh in range(SA):
                        src_ap = bass.AP(xb.tensor, dh * 8 * FS + dh * WINQ,
                                         [[FS, 8], [WIN, NWB], [1, BAND]])
                        dst_ap = bass.AP(o5.tensor, dh * NE,
                                         [[16 * NE, 8], [BAND, NWB], [1, BAND]])
                        nc.sync.dma_start(dst_ap, src_ap)
                    # hop2: per dw group, shift band start by dw
                    o5b = o5bp.tile([128, NEB], bf16, tag="o5b", name="o5b")
                    for dw in range(8):
                        src_ap = bass.AP(o5.tensor, dw * 16 * NE + dw,
                                         [[NE, 16], [BAND, NWB], [1, EB]])
                        dst_ap = bass.AP(o5b.tensor, dw * 16 * NEB,
                                         [[NEB, 16], [EB, NWB], [1, EB]])
                        nc.gpsimd.dma_start(dst_ap, src_ap)
                    o5bs[si] = o5b

                def stage_b(si):
                    """tap extraction + transpose + output for stripe si"""
                    o5b = o5bs.pop(si)
                    o6 = o6p.tile([128, NI2], bf16, tag="o6", name="o6")
                    # tail reads past 2704 hit stale data; the extra psum_t
                    # rows land in ob rows >= TCB that the output never reads
                    src = bass.AP(o5b.tensor, 0,
                                  [[NEB, 128], [EB, NWB], [WINQ, KS], [1, KS]])
                    dst = bass.AP(o6.tensor, 0,
                                  [[NI2, 128], [KK, NWB], [KS, KS], [1, KS]])
                    nc.vector.tensor_copy(dst, src)

                    # tap transpose: psum_t[t, dh*8+dw] via perm matmul
                    ob = obp.tile([128, 2 * SA * W], f32, tag="ob", name="ob")
                    for wb in range(NWB):
                        pt = ptp.tile([128, 256], f32, tag="pt", name="pt")
                        lhsA = bass.AP(o6.tensor, wb * KK,
                                       [[NI2, 128], [1, TCA]])
                        lhsB = bass.AP(o6.tensor, wb * KK + TCA,
                                       [[NI2, 128], [1, TCA]])
                        nc.tensor.matmul(pt[:TCA, 0:128], lhsA, perm[:, :],
                                         start=True, stop=True)
                        nc.tensor.matmul(pt[:TCA, 128:256], lhsB, perm[:, :],
                                         start=True, stop=True)
                        src = bass.AP(pt.tensor, 0,
                                      [[256, TCA], [128, 2], [8, SA], [1, SB]])
                        dst = bass.AP(ob.tensor, wb * SB,
                                      [[2 * SA * W, TCA], [SA * W, 2],
                                       [W, SA], [1, SB]])
                        if wb % 2 == 0:
                            nc.vector.tensor_copy(dst, src)
                        else:
                            nc.scalar.copy(dst, src)

                    # output: 8 DMAs (4 dh-quarters x 2 tap chunks), 2KB runs
                    for i, (tc_n, tbase, obase) in enumerate(
                            ((TCA, 0, 0), (TCB, TCA, SA * W))):
                        for dq in range(4):
                            src = bass.AP(ob.tensor, obase + dq * 4 * W,
                                          [[2 * SA * W, tc_n], [1, 4 * W]])
                            dst = bass.AP(out_d,
                                          tbase * H * W + (si * SA + dq * 4) * W,
                                          [[H * W, tc_n], [1, 4 * W]])
                            if dq % 2 == 0:
                                nc.sync.dma_start(dst, src)
                            else:
                                nc.scalar.dma_start(dst, src)

                load_zt_stripe(0)
                for si in range(NST):
                    if si + 1 < NST:
                        load_zt_stripe(si + 1)
                    if si > 1:
                        stage_b(si - 2)
                    stage_a(si)
                stage_b(NST - 2)
                stage_b(NST - 1)

    nc.compile()
    return nc


def _get_nc():
    if "nc" not in _cache:
        _cache["nc"] = _build()
    return _cache["nc"]


def kernel(z_t: np.ndarray, z_t1: np.ndarray) -> np.ndarray:
    from concourse.bass_utils import run_bass_kernel_spmd

    nc = _get_nc()
    z_t = np.ascontiguousarray(z_t, dtype=np.float32)
    z_t1 = np.ascontiguousarray(z_t1, dtype=np.float32)
    perm_f = _consts()
    B = z_t.shape[0]
    in_maps = [{"z_t": z_t[i], "z_t1": z_t1[i], "perm": perm_f}
               for i in range(B)]
    res = run_bass_kernel_spmd(nc, in_maps, core_ids=list(range(B)))
    return np.stack([res.results[i]["out"] for i in range(B)], axis=0)
